# revision 1
# baseline (speedup 1.0000x reference)
"""Trainium2 Bass kernel for nn_CausalLSTMNodeCell (B=1048576, D=32, H=16, C=3).

Strategy: pure data parallel over the batch across 8 cores, with output rows
organized by residue b mod 3 so the TF-row-major child_r reshape becomes three
extra contiguous-row matmuls. Per 128-row block (K=98, block-diagonal rhs):
  psum[:, 0:32]   = xtB.T @ WB[rho]   (r1, r2 gates)
  psum[:, 32:144] = xtA.T @ WA[rho]   (ifo, n1, n2, r0, a)
All gates (incl. tanh'd "a", via tanh z = 2*sigmoid(2z)-1 with host-prescaled
weights) come out of one Sigmoid op per psum group on ScalarE; elementwise
recombination runs in f32 on VectorE with three muls offloaded to GPSIMD.
Matmul inputs bf16 (host pre-cast + pre-transposed); biases folded in via
ones-rows of the stationary operand. c_prev/child_n ship as bf16; n,h,c are
stored bf16 and upcast on host. Supergroups of 32 blocks are software-
pipelined with LAG=1 (gates phase of s overlaps elementwise of s-1); loads
ride the SP HWDGE ring, stores the ACT ring, to avoid head-of-line blocking.
"""

import numpy as np
import ml_dtypes

B, D, H, C = 1048576, 32, 16, 3
NCORES = 8
R = B // NCORES            # 131072 rows per core
TPB = 342                  # blocks per residue section (padded, 43776>=43691)
TP = TPB * 128             # rows per section
NBLK = 3 * TPB             # blocks per core
SG = 32                    # max blocks per supergroup
GRP = 8                    # blocks per psum group
# supergroup table: per residue section, 10 supergroups of 32 + 1 of 22
SG_TABLE = []              # (blk0, size) pairs
for _rho in range(3):
    _off = _rho * TPB
    for _i in range(TPB // SG):
        SG_TABLE.append((_off + _i * SG, SG))
    if TPB % SG:
        SG_TABLE.append((_off + (TPB // SG) * SG, TPB % SG))
NSG = len(SG_TABLE)
KDIM = 98
bf16 = ml_dtypes.bfloat16

CHILD16 = True             # ship child_n as bf16
CP16 = True                # ship c_prev as bf16
OUT16 = True               # store n,h,c as bf16
GPS_OPS = 3                # how many elementwise ops to push to GPSIMD (0..5)
CGRP = False               # c-chain + tanh_c at psum-group granularity
SPLIT_LOADS = False        # split xt loads in halves for finer deps
XT_BUFS = 3                # buffer depth for xtA/xtB load tiles
AXD_BUFS = 4               # buffer depth for c_prev/child load tiles
CCHAIN_POOL = False        # c-chain (ops 1-3) on GPSIMD instead of DVE
GATES16 = False            # store gate activations as bf16
LAG = 1                    # software-pipeline depth (supergroups)

_NC_CACHE = {}


def _build_w(inputs):
    W = np.zeros((49, 144), np.float32)

    def put(cols, wx, wh, bx, bh):
        W[0:32, cols] = inputs[wx]
        W[32:48, cols] = inputs[wh]
        W[48, cols] = inputs[bx] + inputs[bh]

    put(slice(0, 48), "W_ifo_x", "W_ifo_h", "b_ifo_x", "b_ifo_h")
    put(slice(48, 64), "W_n1_x", "W_n1_h", "b_n1_x", "b_n1_h")
    put(slice(64, 80), "W_n2_x", "W_n2_h", "b_n2_x", "b_n2_h")
    put(slice(80, 96), "W_a_x", "W_a_h", "b_a_x", "b_a_h")
    put(slice(96, 144), "W_r_x", "W_r_h", "b_r_x", "b_r_h")
    return W


def host_prep(inputs):
    x = np.asarray(inputs["inputs"], np.float32)
    hp = np.asarray(inputs["h_prev"], np.float32)
    cp = np.asarray(inputs["c_prev"], np.float32)
    ch = np.asarray(inputs["child_n"], np.float32)
    W = _build_w(inputs)
    Wr = W[:, 96:144]
    xh = np.zeros((B + 1, 49), np.float32)
    xh[:B, 0:32] = x
    xh[:B, 32:48] = hp
    xh[:B, 48] = 1.0
    xh16 = xh.astype(bf16)

    chdt = bf16 if CHILD16 else np.float32
    chall = np.empty((B, 48), chdt)
    chall[:, 0:16] = ch[0].astype(chdt)
    chall[:, 16:32] = ch[1].astype(chdt)
    chall[:, 32:48] = ch[2].astype(chdt)

    cores = []
    for m in range(NCORES):
        xtA = np.zeros((KDIM, 3 * TP), bf16)
        xtB = np.zeros((KDIM, 3 * TP), bf16)
        cpp = np.zeros((3 * TP, 16), bf16)
        chp = np.zeros((3 * TP, 48), chdt)
        WA = np.zeros((3, KDIM, 112), np.float32)
        WB = np.zeros((3, KDIM, 32), np.float32)
        for rho in range(3):
            first = m * R + ((rho - m * R) % 3)
            T = len(range(first, (m + 1) * R, 3))
            sl = slice(rho * TP, rho * TP + TP)
            bidx = first + 3 * np.arange(TP)
            bidx = np.minimum(bidx, B)
            bidx[T:] = B
            xtA[0:49, sl] = xh16[bidx].T
            cpp[sl.start:sl.start + T] = cp[first:(m + 1) * R:3]
            chp[sl.start:sl.start + T] = chall[first:(m + 1) * R:3]
            q = [(k * 16 * B + 16 * first) // 48 for k in range(3)]
            c = [16 * ((k + rho) % 3) for k in range(3)]
            for k, dst, rows in ((0, xtA, slice(49, 98)),
                                 (1, xtB, slice(0, 49)),
                                 (2, xtB, slice(49, 98))):
                qi = np.minimum(q[k] + np.arange(TP), B)
                dst[rows, sl] = xh16[qi].T
            WA[rho, 0:49, 0:48] = W[:, 0:48]            # ifo  -> psum 32:80
            WA[rho, 0:49, 48:64] = W[:, 48:64]          # n1   -> 80:96
            WA[rho, 0:49, 64:80] = W[:, 64:80]          # n2   -> 96:112
            WA[rho, 49:98, 80:96] = Wr[:, c[0]:c[0] + 16]   # r0 -> 112:128
            # a-preact scaled by 2: tanh(z) = 2*sigmoid(2z) - 1, so the a
            # column rides the sigmoid activation with a cheap fixup
            WA[rho, 0:49, 96:112] = 2.0 * W[:, 80:96]   # a    -> 128:144
            WB[rho, 0:49, 0:16] = Wr[:, c[1]:c[1] + 16]     # r1 -> 0:16
            WB[rho, 49:98, 16:32] = Wr[:, c[2]:c[2] + 16]   # r2 -> 16:32
        aux = np.concatenate([cpp, chp], axis=1)        # [3TP, 64] bf16
        aux = np.ascontiguousarray(
            aux.reshape(NBLK, 128, 64).transpose(1, 0, 2))
        cores.append(dict(xta=xtA, xtb=xtB, aux=aux,
                          wa=WA.astype(bf16), wb=WB.astype(bf16)))
    return cores


def build_nc(niter=1, sg_bufs=3):
    import concourse.tile as tile
    from concourse import bacc, mybir

    f32 = mybir.dt.float32
    b16 = mybir.dt.bfloat16
    chdt = b16 if CHILD16 else f32
    cpdt = b16 if CP16 else f32
    odt = b16 if OUT16 else f32
    AF = mybir.ActivationFunctionType

    nc = bacc.Bacc(None, target_bir_lowering=False)
    xta_d = nc.dram_tensor("xta", [KDIM, 3 * TP], b16, kind="ExternalInput")
    xtb_d = nc.dram_tensor("xtb", [KDIM, 3 * TP], b16, kind="ExternalInput")
    wa_d = nc.dram_tensor("wa", [3, KDIM, 112], b16, kind="ExternalInput")
    wb_d = nc.dram_tensor("wb", [3, KDIM, 32], b16, kind="ExternalInput")
    aux_d = nc.dram_tensor("aux", [128, NBLK, 64], b16, kind="ExternalInput")
    res_d = nc.dram_tensor("res", [128, NBLK, 48], odt, kind="ExternalOutput")

    # gate columns in psum/GATES:
    R1, R2 = slice(0, 16), slice(16, 32)
    I, F, O = slice(32, 48), slice(48, 64), slice(64, 80)
    N1, N2, R0 = slice(80, 96), slice(96, 112), slice(112, 128)
    A = slice(128, 144)
    CH0, CH1, CH2 = (slice(16 * i + 16, 16 * i + 32) for i in range(3))
    S0, S1, S2, S3, S4, S5 = (slice(16 * i, 16 * i + 16) for i in range(6))
    RN, RH, RC = (slice(16 * i, 16 * i + 16) for i in range(3))
    ALU = mybir.AluOpType

    with tile.TileContext(nc) as tc:
        with (
            tc.tile_pool(name="wp", bufs=1) as wp,
            tc.tile_pool(name="xtab", bufs=XT_BUFS) as xtabp,
            tc.tile_pool(name="axd", bufs=AXD_BUFS) as axdp,
            tc.tile_pool(name="gates", bufs=sg_bufs) as gatesp,
            tc.tile_pool(name="tmp", bufs=sg_bufs) as tmpp,
            tc.tile_pool(name="res", bufs=sg_bufs) as resp,
            tc.tile_pool(name="ps", bufs=2, space="PSUM") as psp,
        ):
            wa_t = wp.tile([KDIM, 3, 112], b16, tag="wa")
            wb_t = wp.tile([KDIM, 3, 32], b16, tag="wb")
            for rho in range(3):
                nc.sync.dma_start(wa_t[:, rho, :], wa_d[rho])
                nc.sync.dma_start(wb_t[:, rho, :], wb_d[rho])

            V = nc.vector
            G = nc.gpsimd
            E6 = G if GPS_OPS >= 1 else V
            E7 = G if GPS_OPS >= 2 else V
            E8 = G if GPS_OPS >= 3 else V
            E9 = G if GPS_OPS >= 4 else V
            E10 = G if GPS_OPS >= 5 else V
            EC = G if CCHAIN_POOL else V

            def gate_phase(s):
                blk0, sz = SG_TABLE[s]
                rho = blk0 // TPB
                col0 = blk0 * 128
                xta_t = xtabp.tile([KDIM, sz * 128], b16, tag="xta")
                nc.sync.dma_start(xta_t[:], xta_d[:, col0:col0 + sz * 128])
                xtb_t = xtabp.tile([KDIM, sz * 128], b16, tag="xtb")
                nc.sync.dma_start(xtb_t[:], xtb_d[:, col0:col0 + sz * 128])
                auxt = axdp.tile([128, sz, 64], b16, tag="aux")
                nc.sync.dma_start(auxt[:], aux_d[:, blk0:blk0 + sz, :])

                gates = gatesp.tile([128, sz, 144],
                                    b16 if GATES16 else f32, tag="gates")
                tmp = tmpp.tile([128, sz, 96], f32, tag="tmp")
                res = resp.tile([128, sz, 48], odt, tag="res")
                for g in range(-(-sz // GRP)):
                    gsz = min(GRP, sz - g * GRP)
                    ps = psp.tile([128, gsz, 256], f32, tag="ps")
                    for bb in range(gsz):
                        k = g * GRP + bb
                        nc.tensor.matmul(
                            ps[:, bb, 0:32],
                            xtb_t[:, k * 128:(k + 1) * 128],
                            wb_t[:, rho, :])
                        nc.tensor.matmul(
                            ps[:, bb, 32:144],
                            xta_t[:, k * 128:(k + 1) * 128],
                            wa_t[:, rho, :])
                    gsl = slice(g * GRP, g * GRP + gsz)
                    nc.scalar.activation(
                        gates[:, gsl, 0:144], ps[:, :, 0:144], AF.Sigmoid)
                    if CGRP:
                        cchain(gates, auxt, tmp, res, gsl, tanh=True)
                if not CGRP:
                    cchain(gates, auxt, tmp, res, slice(0, sz), tanh=False)
                return (gates, auxt, tmp, res, blk0, sz)

            def cchain(gates, auxt, tmp, res, gs, tanh):
                # a = 2*sigmoid(2z) - 1 fixup (single-input, 2x on DVE)
                V.tensor_scalar(tmp[:, gs, S5], gates[:, gs, A], 2.0,
                                -1.0, ALU.mult, ALU.add)
                EC.tensor_mul(tmp[:, gs, S0], gates[:, gs, I], tmp[:, gs, S5])
                EC.tensor_mul(tmp[:, gs, S1], gates[:, gs, F],
                              auxt[:, gs, 0:16])
                EC.tensor_add(res[:, gs, RC], tmp[:, gs, S0], tmp[:, gs, S1])
                if tanh:
                    nc.scalar.activation(tmp[:, gs, S2], res[:, gs, RC],
                                         AF.Tanh)

            def elem_phase(state):
                gates, auxt, tmp, res, blk0, sz = state
                if not CGRP:
                    nc.scalar.activation(tmp[:, :, S2], res[:, :, RC],
                                         AF.Tanh)
                V.tensor_mul(res[:, :, RH], gates[:, :, O], tmp[:, :, S2])
                E6.tensor_mul(tmp[:, :, S3], gates[:, :, R0],
                              auxt[:, :, CH0])
                E7.tensor_mul(tmp[:, :, S4], gates[:, :, R1],
                              auxt[:, :, CH1])
                E8.tensor_mul(tmp[:, :, S0], gates[:, :, R2],
                              auxt[:, :, CH2])
                E9.tensor_add(tmp[:, :, S1], tmp[:, :, S3], tmp[:, :, S4])
                E10.tensor_add(tmp[:, :, S3], tmp[:, :, S1], tmp[:, :, S0])
                V.tensor_mul(tmp[:, :, S4], gates[:, :, N1], tmp[:, :, S3])
                V.tensor_mul(tmp[:, :, S0], gates[:, :, N2], res[:, :, RH])
                V.tensor_add(res[:, :, RN], tmp[:, :, S4], tmp[:, :, S0])
                # stores ride the ACT HWDGE ring so a store waiting on DVE
                # can't head-of-line-block the next supergroup's loads (SP ring)
                nc.scalar.dma_start(res_d[:, blk0:blk0 + sz, :], res[:])

            total = NSG * niter
            states = {}
            for s in range(total + LAG):
                if s - LAG >= 0 and (s - LAG) in states:
                    elem_phase(states.pop(s - LAG))
                if s < total:
                    states[s] = gate_phase(s % NSG)

    nc.compile()
    return nc


def _get_nc():
    if "nc" not in _NC_CACHE:
        _NC_CACHE["nc"] = build_nc()
    return _NC_CACHE["nc"]


def gather_out(results):
    n = np.empty((B, 16), np.float32)
    h = np.empty((B, 16), np.float32)
    c = np.empty((B, 16), np.float32)
    for m in range(NCORES):
        res = np.asarray(results[m]["res"]).astype(np.float32)
        flat = res.transpose(1, 0, 2).reshape(3 * TP, 48)
        for rho in range(3):
            first = m * R + ((rho - m * R) % 3)
            T = len(range(first, (m + 1) * R, 3))
            seg = flat[rho * TP: rho * TP + T]
            n[first:(m + 1) * R:3] = seg[:, 0:16]
            h[first:(m + 1) * R:3] = seg[:, 16:32]
            c[first:(m + 1) * R:3] = seg[:, 32:48]
    return n, h, c


def make_in_maps(cores):
    return [dict(xta=c["xta"], xtb=c["xtb"], wa=c["wa"], wb=c["wb"],
                 aux=c["aux"]) for c in cores]


def kernel(**inputs):
    from concourse.bass_utils import run_bass_kernel_spmd

    cores = host_prep(inputs)
    nc = _get_nc()
    out = run_bass_kernel_spmd(nc, make_in_maps(cores),
                               core_ids=list(range(NCORES)))
    return gather_out(out.results)



# revision 3
# speedup vs baseline: 2.4448x; 2.4448x over previous
"""Trainium2 Bass kernel for nn_CausalLSTMNodeCell (B=1048576, D=32, H=16, C=3).

Strategy v2: pure data parallel over batch across 8 cores; each core's rows
split into three local sections s (rows b = m*R + s + 3t). Traffic reduction
vs v1: for child k the TF-reshape needs gate rows floor((k*B+b)/3), which is
the SAME row range (+-1 col shift) for all three sections, differing only in
which 16-col block of W_r applies. One gathered copy of [x|h|1] per child
serves all three sections, cutting xh traffic from 4 copies to 2.

Device structure is core-invariant: pair tile P_i carries section i's main
rows (partitions 0:49) and child k_i=(i+m)%3's gather rows (partitions 49:98),
chosen so phi=(k_i*B+m*R)%3 == i. Per triple j and i:
  pair matmul (K=98, N=128): psum block i cols 0:128 =
      [main gates of section i (96) | r-gates of child grp i, 2 "A" sections]
  B matmul (K=98 zero-top, N=16): block i cols 128:144 = remaining section.
Psum triple layout [3, 176] (last 32 pad); gates SBUF [SZ, 3, 144]. One
Sigmoid per psum group covers all gates (tanh'd "a" rides it via
2*sigmoid(2z)-1, weights pre-scaled). Elementwise on VectorE/GPSIMD; aux
(c_prev, child groups) bf16; outputs n,h,c bf16, upcast on host.
"""

import numpy as np
import ml_dtypes

B, D, H, C = 1048576, 32, 16, 3
NCORES = 8
R = B // NCORES            # 131072 rows per core
TPB = 342                  # triple-blocks per section (43776 >= 43691)
TP = TPB * 128
TPW = TP + 128             # +1 block for the delta col shifts
SZ = 18                    # triples per supergroup (342 = 19*18)
GRP = 3                    # triples per psum group
NSG = TPB // SZ            # 19
KDIM = 98
bf16 = ml_dtypes.bfloat16

# core-invariant plan: phi=i => delta pattern over sections
DA = (0, 0, 1)             # pair-matmul col shift per block i
DB = (0, 1, 0)             # B-matmul col shift per block i
SA = ((0, 1), (0, 1), (1, 2))   # the two "A" sections of block i
SB = (2, 2, 0)                  # the "B" section of block i
# r-gate col within block i for section s (96 + 16*pos, B at 128)
RCOL = [[96, 96, 128], [112, 112, 96], [128, 128, 112]]  # [s][i]

XT_BUFS = 3
AXD_BUFS = 3
SG_BUFS = 3
GATES_BF16 = True          # gates tile bf16 (DVE 2x mode)
TMP_BF16 = True            # tmp tile bf16 (DVE 2x mode)
GPS_RMUL2 = False          # rmul block-2 trio on GPSIMD
GPS_N2H = False            # n2h on GPSIMD
GPS_FC = False             # f*c_prev on GPSIMD
STORE_GPS = True           # res store trigger on Pool queue (not ACT)
AUX_ACT = True             # aux load trigger on ACT HWDGE ring (parallel to SP)
SPLIT_P = 0                # 0: P all on SP; 1: P[2] on DVE; 2: P[2] on ACT
LAG = 1
# timing-only ablation flags (break numerics; for bottleneck attribution)
ABL_NO_B = False           # skip the two B matmuls per triple
ABL_NO_TANH = False        # skip tanh
ABL_DVE_LITE = False       # skip the r-combination DVE ops
ABL_NO_AUX = False         # skip aux loads
ABL_NO_STORE = False       # skip res stores

_NC_CACHE = {}


def _build_w(inputs):
    """W [49, 144]: rows 0:32 x-w, 32:48 h-w, 48 bias. Cols: ifo 0:48,
    n1 48:64, n2 64:80, a(x2) 80:96, r 96:144."""
    W = np.zeros((49, 144), np.float32)

    def put(cols, wx, wh, bx, bh, scale=1.0):
        W[0:32, cols] = scale * np.asarray(inputs[wx], np.float32)
        W[32:48, cols] = scale * np.asarray(inputs[wh], np.float32)
        W[48, cols] = scale * (np.asarray(inputs[bx], np.float32)
                               + np.asarray(inputs[bh], np.float32))

    put(slice(0, 48), "W_ifo_x", "W_ifo_h", "b_ifo_x", "b_ifo_h")
    put(slice(48, 64), "W_n1_x", "W_n1_h", "b_n1_x", "b_n1_h")
    put(slice(64, 80), "W_n2_x", "W_n2_h", "b_n2_x", "b_n2_h")
    put(slice(80, 96), "W_a_x", "W_a_h", "b_a_x", "b_a_h", scale=2.0)
    put(slice(96, 144), "W_r_x", "W_r_h", "b_r_x", "b_r_h")
    return W


def host_prep(inputs):
    x = np.asarray(inputs["inputs"], np.float32)
    hp = np.asarray(inputs["h_prev"], np.float32)
    cp = np.asarray(inputs["c_prev"], np.float32)
    ch = np.asarray(inputs["child_n"], np.float32)
    W = _build_w(inputs)
    Wr = W[:, 96:144]
    xh = np.zeros((B + 1, 49), np.float32)
    xh[:B, 0:32] = x
    xh[:B, 32:48] = hp
    xh[:B, 48] = 1.0
    xh16 = xh.astype(bf16)

    cp16 = np.concatenate([cp.astype(bf16), np.zeros((1, 16), bf16)])
    ch16 = [np.concatenate([ch[k].astype(bf16), np.zeros((1, 16), bf16)])
            for k in range(3)]

    # stationary weights: core-invariant; i=0's B cols fold into its pair
    # stationary (delta_A == delta_B for block 0), so wp is [3, KDIM, 144]
    wp = np.zeros((3, KDIM, 144), np.float32)
    wb = np.zeros((3, KDIM, 16), np.float32)
    for i in range(3):
        wp[i, 0:49, 0:96] = W[:, 0:96]
        ca0 = 16 * ((i + SA[i][0]) % 3)
        ca1 = 16 * ((i + SA[i][1]) % 3)
        cb = 16 * ((i + SB[i]) % 3)
        wp[i, 49:98, 96:112] = Wr[:, ca0:ca0 + 16]
        wp[i, 49:98, 112:128] = Wr[:, ca1:ca1 + 16]
        if i == 0:
            wp[i, 49:98, 128:144] = Wr[:, cb:cb + 16]
        else:
            wb[i, 49:98, 0:16] = Wr[:, cb:cb + 16]
    wp16, wb16 = wp.astype(bf16), wb.astype(bf16)

    cores = []
    u = np.arange(TPW)
    t_tp = np.arange(TP)
    for m in range(NCORES):
        pt = np.zeros((3, KDIM, TPW), bf16)
        aux = np.empty((TPB, 3, 128, 64), bf16)
        for i in range(3):
            first = m * R + i
            T = len(range(first, (m + 1) * R, 3))
            k = (i + m) % 3
            qmin = (k * B + m * R) // 3
            # main half: col u = xh[first + 3*(u - DA[i])]
            t = u - DA[i]
            bidx = np.where((t < 0) | (t >= T), B, first + 3 * t)
            pt[i, 0:49, :] = xh16[bidx].T
            # child half: col u = xh[qmin + u]
            gidx = np.minimum(qmin + u, B)
            pt[i, 49:98, :] = xh16[gidx].T
            # aux for section i: [cp | ch_grp0 | ch_grp1 | ch_grp2]
            bidx2 = np.where(t_tp < T, first + 3 * t_tp, B)
            sec = np.empty((TP, 64), bf16)
            sec[:, 0:16] = cp16[bidx2]
            for g in range(3):
                sec[:, 16 + 16 * g:32 + 16 * g] = ch16[(g + m) % 3][bidx2]
            aux[:, i, :, :] = sec.reshape(TPB, 128, 64)
        auxt = np.ascontiguousarray(aux.transpose(2, 0, 1, 3))  # [128,TPB,3,64]
        pall = np.ascontiguousarray(pt.transpose(1, 0, 2))  # [KDIM, 3, TPW]
        cores.append(dict(pall=pall, wp=wp16, wb=wb16, aux=auxt))
    return cores


def build_nc(niter=1):
    import concourse.tile as tile
    from concourse import bacc, mybir

    f32 = mybir.dt.float32
    b16 = mybir.dt.bfloat16
    AF = mybir.ActivationFunctionType
    ALU = mybir.AluOpType

    nc = bacc.Bacc(None, target_bir_lowering=False)
    p_d = nc.dram_tensor("pall", [KDIM, 3, TPW], b16, kind="ExternalInput")
    wp_d = nc.dram_tensor("wp", [3, KDIM, 144], b16, kind="ExternalInput")
    wb_d = nc.dram_tensor("wb", [3, KDIM, 16], b16, kind="ExternalInput")
    aux_d = nc.dram_tensor("aux", [128, TPB, 3, 64], b16, kind="ExternalInput")
    res_d = nc.dram_tensor("res", [128, TPB, 3, 48], b16,
                           kind="ExternalOutput")

    # tmp scratch slices (f32), [128, SZ, 3, 96]
    S0, S1, S2, S3, S4, S5 = (slice(16 * i, 16 * i + 16) for i in range(6))
    # gate cols within a 144-block
    GI, GF, GO = slice(0, 16), slice(16, 32), slice(32, 48)
    GN1, GN2, GA = slice(48, 64), slice(64, 80), slice(80, 96)

    with tile.TileContext(nc) as tc:
        with (
            tc.tile_pool(name="wpool", bufs=1) as wpool,
            tc.tile_pool(name="xtab", bufs=XT_BUFS) as xtabp,
            tc.tile_pool(name="axd", bufs=AXD_BUFS) as axdp,
            tc.tile_pool(name="gates", bufs=SG_BUFS) as gatesp,
            tc.tile_pool(name="tmp", bufs=SG_BUFS) as tmpp,
            tc.tile_pool(name="res", bufs=SG_BUFS) as resp,
            tc.tile_pool(name="ps", bufs=2, space="PSUM") as psp,
        ):
            wp_t = wpool.tile([KDIM, 3, 144], b16, tag="wp")
            wb_t = wpool.tile([KDIM, 3, 16], b16, tag="wb")
            for i in range(3):
                nc.sync.dma_start(wp_t[:, i, :], wp_d[i])
                nc.sync.dma_start(wb_t[:, i, :], wb_d[i])

            V = nc.vector
            G = nc.gpsimd
            E_RM2 = G if GPS_RMUL2 else V
            E_N2H = G if GPS_N2H else V
            E_FC = G if GPS_FC else V

            def gate_phase(sg):
                blk0 = sg * SZ
                sz = min(SZ, TPB - blk0)
                c0 = blk0 * 128
                cw = sz * 128 + 128
                pt = xtabp.tile([KDIM, 3, cw], b16, tag="pall")
                if SPLIT_P == 0:
                    nc.sync.dma_start(pt[:], p_d[:, :, c0:c0 + cw])
                else:
                    PE2 = nc.gpsimd if SPLIT_P == 1 else nc.scalar
                    nc.sync.dma_start(pt[:, 0:2, :], p_d[:, 0:2, c0:c0 + cw])
                    PE2.dma_start(pt[:, 2, :], p_d[:, 2, c0:c0 + cw])
                auxt = axdp.tile([128, sz, 3, 64], b16, tag="aux")
                AUXE = nc.scalar if AUX_ACT else nc.sync
                if ABL_NO_AUX:
                    # tiny load keeps the tile written; kills the traffic
                    AUXE.dma_start(auxt[:, 0:1, :, :],
                                   aux_d[:, blk0:blk0 + 1, :, :])
                else:
                    AUXE.dma_start(auxt[:], aux_d[:, blk0:blk0 + sz, :, :])

                gates = gatesp.tile([128, sz, 3, 144],
                                    b16 if GATES_BF16 else f32, tag="gates")
                tmp = tmpp.tile([128, sz, 3, 96],
                                b16 if TMP_BF16 else f32, tag="tmp")
                res = resp.tile([128, sz, 3, 48], b16, tag="res")
                for g in range(-(-sz // GRP)):
                    gsz = min(GRP, sz - g * GRP)
                    ps = psp.tile([128, gsz, 3, 176], f32, tag="ps")
                    for jj in range(gsz):
                        j = g * GRP + jj
                        for i in range(3):
                            nw = 144 if i == 0 else 128
                            nc.tensor.matmul(
                                ps[:, jj, i, 0:nw],
                                pt[:, i, j * 128 + DA[i]:
                                   j * 128 + DA[i] + 128],
                                wp_t[:, i, 0:nw])
                            if i != 0 and not ABL_NO_B:
                                nc.tensor.matmul(
                                    ps[:, jj, i, 128:144],
                                    pt[:, i, j * 128 + DB[i]:
                                       j * 128 + DB[i] + 128],
                                    wb_t[:, i, :])
                    gsl = slice(g * GRP, g * GRP + gsz)
                    nc.scalar.activation(
                        gates[:, gsl, :, :], ps[:, :, :, 0:144], AF.Sigmoid)
                return (gates, auxt, tmp, res, blk0, sz)

            def elem_phase(state):
                gates, auxt, tmp, res, blk0, sz = state
                ALL = slice(None)
                g3 = (ALL, ALL, ALL)
                # a = 2*sigmoid(2z) - 1
                V.tensor_scalar(tmp[:, :, :, S5], gates[:, :, :, GA],
                                2.0, -1.0, ALU.mult, ALU.add)
                V.tensor_mul(tmp[:, :, :, S0], gates[:, :, :, GI],
                             tmp[:, :, :, S5])
                E_FC.tensor_mul(tmp[:, :, :, S1], gates[:, :, :, GF],
                                auxt[:, :, :, 0:16])
                V.tensor_add(res[:, :, :, 32:48], tmp[:, :, :, S0],
                             tmp[:, :, :, S1])
                if not ABL_NO_TANH:
                    nc.scalar.activation(tmp[:, :, :, S2],
                                         res[:, :, :, 32:48], AF.Tanh)
                V.tensor_mul(res[:, :, :, 16:32], gates[:, :, :, GO],
                             tmp[:, :, :, S2])
                # r-gate * child products: blocks 0,1 are s-ordered 96:144
                if not ABL_DVE_LITE:
                    V.tensor_mul(tmp[:, :, :, S3], gates[:, :, 0, 96:144],
                                 auxt[:, :, :, 16:32])
                    V.tensor_mul(tmp[:, :, :, S4], gates[:, :, 1, 96:144],
                                 auxt[:, :, :, 32:48])
                    for s in range(3):
                        E_RM2.tensor_mul(
                            tmp[:, :, s, S0], gates[:, :, 2, RCOL[s][2]:
                                                    RCOL[s][2] + 16],
                            auxt[:, :, s, 48:64])
                    V.tensor_add(tmp[:, :, :, S5], tmp[:, :, :, S3],
                                 tmp[:, :, :, S4])
                    V.tensor_add(tmp[:, :, :, S3], tmp[:, :, :, S5],
                                 tmp[:, :, :, S0])
                V.tensor_mul(tmp[:, :, :, S4], gates[:, :, :, GN1],
                             tmp[:, :, :, S3])
                E_N2H.tensor_mul(tmp[:, :, :, S0], gates[:, :, :, GN2],
                                 res[:, :, :, 16:32])
                V.tensor_add(res[:, :, :, 0:16], tmp[:, :, :, S4],
                             tmp[:, :, :, S0])
                # stores ride the Pool queue (loads use SP) to keep the ACT
                # engine free for sigmoids and avoid head-of-line blocking
                if ABL_NO_STORE:
                    pass
                elif STORE_GPS:
                    nc.gpsimd.dma_start(res_d[:, blk0:blk0 + sz, :, :], res[:])
                else:
                    nc.scalar.dma_start(res_d[:, blk0:blk0 + sz, :, :], res[:])

            def one_pass():
                state = gate_phase(0)
                for sg in range(1, NSG):
                    nstate = gate_phase(sg)
                    elem_phase(state)
                    state = nstate
                elem_phase(state)

            if niter == 1:
                one_pass()
            else:
                with tc.For_i(0, niter):
                    one_pass()

    nc.compile()
    return nc


def _get_nc():
    if "nc" not in _NC_CACHE:
        _NC_CACHE["nc"] = build_nc()
    return _NC_CACHE["nc"]


def gather_out(results):
    n = np.empty((B, 16), np.float32)
    h = np.empty((B, 16), np.float32)
    c = np.empty((B, 16), np.float32)
    for m in range(NCORES):
        res = np.asarray(results[m]["res"]).astype(np.float32)
        # [128, TPB, 3, 48] -> per section s: [TP, 48]
        for s in range(3):
            first = m * R + s
            T = len(range(first, (m + 1) * R, 3))
            flat = res[:, :, s, :].transpose(1, 0, 2).reshape(TP, 48)
            n[first:(m + 1) * R:3] = flat[:T, 0:16]
            h[first:(m + 1) * R:3] = flat[:T, 16:32]
            c[first:(m + 1) * R:3] = flat[:T, 32:48]
    return n, h, c


def make_in_maps(cores):
    return [dict(pall=c["pall"], wp=c["wp"], wb=c["wb"], aux=c["aux"])
            for c in cores]


def kernel(**inputs):
    from concourse.bass_utils import run_bass_kernel_spmd

    cores = host_prep(inputs)
    nc = _get_nc()
    out = run_bass_kernel_spmd(nc, make_in_maps(cores),
                               core_ids=list(range(NCORES)))
    return gather_out(out.results)
